# revision 1
# baseline (speedup 1.0000x reference)
"""Causal attention (B=4, T=4096, D=768) on 8 trn2 NeuronCores.

Sharding: 2 cores per batch element. Core c: batch b = c % 4, parity a = c // 4.
Core (b, a) owns query blocks {4u + 2a, 4u + 2a + 1 : u = 0..7} of its batch
(zigzag assignment), so every core runs an IDENTICAL SPMD program: query pair
u iterates over j-blocks [0, 4u+4), with the trailing 4 j-blocks masked via
per-core additive mask tiles supplied as input data.

Each core computes the full K/V of its batch (rows [0:2048) SBUF-resident,
rows [2048:4096) spilled to DRAM scratch and re-streamed during attention).
The host supplies x / xq / weights already transposed ([D, T] layouts) so the
kernel needs no on-chip transposes; all matmuls run in float32r (full PE rate
at free-dim >= 256, ~tf32 operand rounding).
"""

import sys

for p in ("/opt/trn_rl_repo", "/root/.axon_site/_ro/trn_rl_repo"):
    if p not in sys.path:
        sys.path.insert(0, p)

import numpy as np

B, T, D = 4, 4096, 768
DC = D // 128            # d (contraction) chunks
OC = D // 128            # o (output) chunks
NQ = 2048                # local query rows per core
NPAIR = 8                # query pairs (256 rows each)
KVLOW = 2048             # SBUF-resident kv rows
NJLOW = KVLOW // 128
SCALE = 1.0 / float(np.sqrt(D))
NEG = -1.0e9

_COMPILED = None


def build_program():
    import concourse.tile as tile
    from concourse import bacc, mybir

    f32 = mybir.dt.float32
    f32r = mybir.dt.float32r
    Exp = mybir.ActivationFunctionType.Exp

    nc = bacc.Bacc()
    xT_d = nc.declare_dram_parameter("xT", [D, T], f32r, isOutput=False)
    xqT_d = nc.declare_dram_parameter("xqT", [D, NQ], f32r, isOutput=False)
    wqT_d = nc.declare_dram_parameter("wqT", [D, D], f32r, isOutput=False)
    wkT_d = nc.declare_dram_parameter("wkT", [D, D], f32r, isOutput=False)
    wvT_d = nc.declare_dram_parameter("wvT", [D, D], f32r, isOutput=False)
    masks_d = nc.declare_dram_parameter("masks", [4, 128, 256], f32, isOutput=False)
    out_d = nc.declare_dram_parameter("out", [NQ, D], f32, isOutput=True)

    kThi_d = nc.dram_tensor("kThi", [DC, 128, T - KVLOW], f32r)
    vHi_d = nc.dram_tensor("vHi", [T - KVLOW, D + 2], f32r)

    mm = nc.tensor.matmul

    with tile.TileContext(nc) as tc:
        with tc.tile_pool(name="res", bufs=1) as res:
            masks = res.tile([128, 4, 256], f32)
            ones1 = res.tile([128, 2], f32)
            nc.vector.memset(ones1[:, 0:1], 1.0)
            nc.vector.memset(ones1[:, 1:2], 0.0)
            kT = res.tile([128, DC, KVLOW], f32r)        # [o, oc, t]
            vLow = res.tile([128, NJLOW, D + 2], f32r)   # [t, jb, o+2]

            # ---- Phase 1+2: K/V projection over all 4096 rows
            with (
                tc.tile_pool(name="wkv", bufs=1) as wkvp,
                tc.tile_pool(name="p12x", bufs=3) as p12x,
                tc.tile_pool(name="stage", bufs=3) as stage,
                tc.tile_pool(name="ps_k", bufs=3, space="PSUM") as ps_k,
                tc.tile_pool(name="ps_v", bufs=2, space="PSUM") as ps_v,
            ):
                wkT = wkvp.tile([128, DC, D], f32r)
                wvT = wkvp.tile([128, DC, D], f32r)
                for dc in range(DC):
                    nc.default_dma_engine.dma_start(
                        out=wkT[:, dc, :], in_=wkT_d[dc * 128:(dc + 1) * 128, :]
                    )

                for tch in range(T // 512):
                    t0 = tch * 512
                    xTc = p12x.tile([128, DC, 512], f32r, tag="xTc")
                    for dc in range(DC):
                        nc.default_dma_engine.dma_start(
                            out=xTc[:, dc, :],
                            in_=xT_d[dc * 128:(dc + 1) * 128, t0:t0 + 512],
                        )
                    # K^T chunk: [o 128, t 512] per o-chunk
                    for oc in range(OC):
                        pk = ps_k.tile([128, 512], f32, tag="pk")
                        for dc in range(DC):
                            mm(pk, wkT[:, dc, oc * 128:(oc + 1) * 128],
                               xTc[:, dc, :],
                               start=(dc == 0), stop=(dc == DC - 1))
                        if t0 < KVLOW:
                            nc.vector.tensor_copy(kT[:, oc, t0:t0 + 512], pk)
                        else:
                            st = stage.tile([128, 512], f32r, tag="kst")
                            nc.scalar.copy(st, pk)
                            nc.gpsimd.dma_start(
                                out=kThi_d[oc, :, t0 - KVLOW:t0 - KVLOW + 512],
                                in_=st,
                            )
                    if tch == 0:
                        for dc in range(DC):
                            nc.default_dma_engine.dma_start(
                                out=wvT[:, dc, :],
                                in_=wvT_d[dc * 128:(dc + 1) * 128, :],
                            )
                    # V chunk: [t 128, o 768] per 128-row subchunk
                    for s in range(4):
                        pv = ps_v.tile([128, D], f32, tag="pv")
                        for dc in range(DC):
                            for n0, n1 in ((0, 512), (512, D)):
                                mm(pv[:, n0:n1],
                                   xTc[:, dc, s * 128:(s + 1) * 128],
                                   wvT[:, dc, n0:n1],
                                   start=(dc == 0), stop=(dc == DC - 1))
                        jb = (t0 + s * 128) // 128
                        if t0 < KVLOW:
                            nc.vector.tensor_copy(vLow[:, jb, 0:D], pv)
                            nc.vector.tensor_copy(vLow[:, jb, D:D + 2], ones1)
                        else:
                            st = stage.tile([128, D + 2], f32r, tag="vst")
                            nc.scalar.copy(st[:, 0:D], pv)
                            nc.vector.tensor_copy(st[:, D:D + 2], ones1)
                            r0 = t0 + s * 128 - KVLOW
                            nc.gpsimd.dma_start(
                                out=vHi_d[r0:r0 + 128, :], in_=st
                            )

            # ---- Phase 3: Q projection for local query rows
            qtp = tc.alloc_tile_pool(name="qtp", bufs=1)
            qT = qtp.tile([128, DC, NQ], f32r, name="qT")
            with tc.tile_pool(name="wq2", bufs=1) as wq2, \
                 tc.tile_pool(name="p3x", bufs=3) as p3x, \
                 tc.tile_pool(name="ps_q", bufs=3, space="PSUM") as ps_q:
                wqT = wq2.tile([128, DC, D], f32r, name="wqT")
                for dc in range(DC):
                    nc.default_dma_engine.dma_start(
                        out=wqT[:, dc, :], in_=wqT_d[dc * 128:(dc + 1) * 128, :]
                    )
                for tch in range(NQ // 512):
                    t0 = tch * 512
                    xTc = p3x.tile([128, DC, 512], f32r, tag="xTc")
                    for dc in range(DC):
                        nc.default_dma_engine.dma_start(
                            out=xTc[:, dc, :],
                            in_=xqT_d[dc * 128:(dc + 1) * 128, t0:t0 + 512],
                        )
                    for oc in range(OC):
                        pq = ps_q.tile([128, 512], f32, tag="pq")
                        for dc in range(DC):
                            mm(pq, wqT[:, dc, oc * 128:(oc + 1) * 128],
                               xTc[:, dc, :],
                               start=(dc == 0), stop=(dc == DC - 1))
                        nc.vector.tensor_copy(qT[:, oc, t0:t0 + 512], pq)

            nc.default_dma_engine.dma_start(
                out=masks, in_=masks_d.rearrange("m p f -> p m f")
            )

            # ---- Phase 4: attention (LAG-pipelined)
            LAG = 2
            sched = [(u, jj) for u in range(NPAIR) for jj in range(4 * u + 4)]
            with (
                tc.tile_pool(name="hist", bufs=6) as hist,
                tc.tile_pool(name="expp", bufs=4) as expp,
                tc.tile_pool(name="outp", bufs=3) as outp,
                tc.tile_pool(name="ps_av", bufs=1, space="PSUM") as ps_av,
                tc.tile_pool(name="ps_s", bufs=4, space="PSUM") as ps_s,
            ):
                av_tiles = {}
                pending = []

                def emit_scores(u, jj):
                    if jj < NJLOW:
                        kslab = kT[:, :, jj * 128:(jj + 1) * 128]
                        vslab = vLow[:, jj, :]
                    else:
                        khi = hist.tile([128, DC, 128], f32r, tag="khi",
                                        name=f"khi{u}_{jj}")
                        nc.default_dma_engine.dma_start(
                            out=khi,
                            in_=kThi_d.rearrange("c p t -> p c t")[
                                :, :, (jj - NJLOW) * 128:(jj - NJLOW + 1) * 128
                            ],
                        )
                        vhi = hist.tile([128, D + 2], f32r, tag="vhi",
                                        name=f"vhi{u}_{jj}")
                        nc.default_dma_engine.dma_start(
                            out=vhi,
                            in_=vHi_d[(jj - NJLOW) * 128:(jj - NJLOW + 1) * 128, :],
                        )
                        kslab, vslab = khi, vhi
                    ps = ps_s.tile([128, 256], f32, tag="ps", name=f"ps{u}_{jj}")
                    for oc in range(OC):
                        mm(ps, kslab[:, oc, :], qT[:, oc, u * 256:(u + 1) * 256],
                           start=(oc == 0), stop=(oc == OC - 1))
                    m = jj - 4 * u
                    if m >= 0:
                        nc.vector.tensor_add(ps, ps, masks[:, m, :])
                    ex = expp.tile([128, 256], f32r, tag="ex", name=f"ex{u}_{jj}")
                    nc.scalar.activation(ex, ps, Exp, scale=SCALE)
                    return (u, jj, ex, vslab)

                def emit_av(u, jj, ex, vslab):
                    njb = 4 * u + 4
                    if jj == 0:
                        av_tiles[u] = [
                            ps_av.tile([128, 1024], f32, tag=f"av{g}",
                                       name=f"av{u}_{g}")
                            for g in (0, 1)
                        ]
                    av = av_tiles[u]
                    for g in (0, 1):
                        for n0, n1 in ((0, 512), (512, D + 2)):
                            mm(av[g][:, n0:n1], ex[:, g * 128:(g + 1) * 128],
                               vslab[:, n0:n1],
                               start=(jj == 0), stop=(jj == njb - 1))
                    if jj == njb - 1:
                        for g in (0, 1):
                            rec = outp.tile([128, 1], f32, tag="rec",
                                            name=f"rec{u}_{g}")
                            nc.vector.reciprocal(rec, av[g][:, D:D + 1])
                            ot = outp.tile([128, D], f32, tag="ot",
                                           name=f"ot{u}_{g}")
                            nc.scalar.mul(ot, av[g][:, 0:D], rec)
                            r0 = (2 * u + g) * 128
                            nc.default_dma_engine.dma_start(
                                out=out_d[r0:r0 + 128, :], in_=ot
                            )
                        del av_tiles[u]

                for idx in range(len(sched) + LAG):
                    if idx < len(sched):
                        pending.append(emit_scores(*sched[idx]))
                    if idx >= LAG:
                        emit_av(*pending.pop(0))
            qtp.release()
    nc.finalize()
    return nc


def _build_masks(a: int) -> np.ndarray:
    """Additive pre-softmax masks for the last 4 j-blocks of each pair."""
    keep = np.triu(np.ones((128, 128), dtype=bool))  # keep iff j(p) <= i(f)
    P0 = np.zeros((128, 256), dtype=np.float32)
    P1 = np.zeros((128, 256), dtype=np.float32)
    P1[:, :128] = np.where(keep, 0.0, NEG)
    P2 = np.full((128, 256), NEG, dtype=np.float32)
    P2[:, 128:] = np.where(keep, 0.0, NEG)
    P3 = np.full((128, 256), NEG, dtype=np.float32)
    if a == 0:
        return np.stack([P1, P2, P3, P3])
    return np.stack([P0, P0, P1, P2])


def _local_blocks(a: int):
    """Global 128-row block index for each local block L = 0..15."""
    return [4 * (L // 2) + 2 * a + (L % 2) for L in range(16)]


def build_in_maps(x, W_q, W_k, W_v):
    x = np.ascontiguousarray(x, dtype=np.float32)
    wqT = np.ascontiguousarray(np.asarray(W_q, dtype=np.float32).T)
    wkT = np.ascontiguousarray(np.asarray(W_k, dtype=np.float32).T)
    wvT = np.ascontiguousarray(np.asarray(W_v, dtype=np.float32).T)
    masks = [_build_masks(a) for a in (0, 1)]
    xT = np.ascontiguousarray(x.transpose(0, 2, 1))  # [B, D, T]

    in_maps = []
    for c in range(8):
        b, a = c % 4, c // 4
        xTb = xT[b]
        xqT = np.concatenate(
            [xTb[:, gb * 128:(gb + 1) * 128] for gb in _local_blocks(a)], axis=1
        )
        in_maps.append(
            {
                "xT": xTb,
                "xqT": np.ascontiguousarray(xqT),
                "wqT": wqT,
                "wkT": wkT,
                "wvT": wvT,
                "masks": masks[a],
            }
        )
    return in_maps


def last_in_maps(inputs):
    return build_in_maps(
        inputs["x"], inputs["W_q"], inputs["W_k"], inputs["W_v"]
    )


def kernel(x, W_q, W_k, W_v):
    global _COMPILED
    from concourse.bass_utils import run_bass_kernel_spmd

    if _COMPILED is None:
        _COMPILED = build_program()
    nc = _COMPILED

    in_maps = build_in_maps(x, W_q, W_k, W_v)
    res = run_bass_kernel_spmd(nc, in_maps, list(range(8)))
    out = np.empty((B, T, D), dtype=np.float32)
    for c in range(8):
        b, a = c % 4, c // 4
        oc_loc = res.results[c]["out"]
        for L, gb in enumerate(_local_blocks(a)):
            out[b, gb * 128:(gb + 1) * 128] = oc_loc[L * 128:(L + 1) * 128]
    return out



# revision 5
# speedup vs baseline: 1.3223x; 1.3223x over previous
"""Causal attention (B=4, T=4096, D=768) on 8 trn2 NeuronCores.

Sharding: 2 cores per batch element. Core c: batch b = c % 4, parity a = c // 4.
Core (b, a) owns query blocks {4u + 2a, 4u + 2a + 1 : u = 0..7} (zigzag), so all
cores run one SPMD program with equal work.

Transfer-minimized formulation (the graded time is dominated by host<->device
bytes, not device compute):
  - scores = x (Wq^T Wk) x^T / sqrt(D): the host pre-multiplies M = Wq^T Wk, so
    no K projection exists and keys are raw xT (already an input).
  - All device inputs/outputs are bfloat16 (rel-err budget 2e-2; measured
    ~6e-3). PSUM accumulation stays f32.
  - Per-core xT is sent with column blocks permuted so each core's query pair u
    sits at fixed positions {4u+2, 4u+3}; the key j-loop runs over permuted
    positions 0..4u+3 with the trailing 4 handled by per-core additive mask
    tiles. This removes the separate pre-gathered xq input.
  - bf16 K/V (V = x Wv^T, 4096 rows) fits entirely in SBUF: no DRAM spill.
"""

import sys

for p in ("/opt/trn_rl_repo", "/root/.axon_site/_ro/trn_rl_repo"):
    if p not in sys.path:
        sys.path.insert(0, p)

import numpy as np

B, T, D = 4, 4096, 768
DC = D // 128             # contraction chunks
OC = D // 128             # output chunks
NQ = 2048                 # local query rows per core
NPAIR = 8                 # query pairs (256 rows each)
NJB = T // 128            # key blocks
SCALE = 1.0 / float(np.sqrt(D))
NEG = -1.0e9

_COMPILED = None


def build_program():
    import concourse.tile as tile
    from concourse import bacc, mybir

    f32 = mybir.dt.float32
    bf16 = mybir.dt.bfloat16
    Exp = mybir.ActivationFunctionType.Exp

    nc = bacc.Bacc()
    xT_d = nc.declare_dram_parameter("xT", [D, T], bf16, isOutput=False)
    m_d = nc.declare_dram_parameter("m", [D, D], bf16, isOutput=False)
    wvT_d = nc.declare_dram_parameter("wvT", [D, D], bf16, isOutput=False)
    masks_d = nc.declare_dram_parameter("masks", [128, 4, 256], bf16, isOutput=False)
    out_d = nc.declare_dram_parameter("out", [NQ, D], bf16, isOutput=True)

    mm = nc.tensor.matmul

    with tile.TileContext(nc) as tc:
        with tc.tile_pool(name="res", bufs=1) as res:
            xT = res.tile([128, DC, T], bf16)        # [d, dc, t] permuted cols
            vV = res.tile([128, NJB, D + 2], bf16)   # [t, jb, o + (1,0)]
            gqT = res.tile([128, OC, NQ], bf16)      # [o, oc, q]
            mT = res.tile([128, DC, D], bf16)        # M = Wq^T Wk, [d, dc, o]
            wvT = res.tile([128, DC, D], bf16)       # Wv^T, [d, dc, o]
            masksb = res.tile([128, 4, 256], bf16)
            masks = res.tile([128, 4, 256], f32)
            ones1 = res.tile([128, 2], bf16)
            nc.vector.memset(ones1[:, 0:1], 1.0)
            nc.vector.memset(ones1[:, 1:2], 0.0)

            for dc in range(DC):
                nc.default_dma_engine.dma_start(
                    out=xT[:, dc, :], in_=xT_d[dc * 128:(dc + 1) * 128, :]
                )
            for dc in range(DC):
                nc.default_dma_engine.dma_start(
                    out=mT[:, dc, :], in_=m_d[dc * 128:(dc + 1) * 128, :]
                )
            for dc in range(DC):
                nc.default_dma_engine.dma_start(
                    out=wvT[:, dc, :], in_=wvT_d[dc * 128:(dc + 1) * 128, :]
                )
            nc.default_dma_engine.dma_start(out=masksb, in_=masks_d[:, :, :])
            nc.vector.tensor_copy(masks, masksb)

            # ---- Phase G: gqT[o, q] = sum_d M[d, o] * xq[d, q]
            with tc.tile_pool(name="ps_g", bufs=3, space="PSUM") as ps_g:
                for u in range(NPAIR):
                    q0 = (4 * u + 2) * 128
                    for oc in range(OC):
                        pg = ps_g.tile([128, 256], f32, tag="pg")
                        for dc in range(DC):
                            mm(pg, mT[:, dc, oc * 128:(oc + 1) * 128],
                               xT[:, dc, q0:q0 + 256],
                               start=(dc == 0), stop=(dc == DC - 1))
                        nc.scalar.copy(gqT[:, oc, u * 256:(u + 1) * 256], pg)

            # ---- Phase V: V[t, o] = sum_d x[t, d] * Wv[o, d], + ones column
            with tc.tile_pool(name="ps_v", bufs=3, space="PSUM") as ps_v:
                for jb in range(NJB):
                    pv = ps_v.tile([128, D], f32, tag="pv")
                    for dc in range(DC):
                        for n0, n1 in ((0, 512), (512, D)):
                            mm(pv[:, n0:n1],
                               xT[:, dc, jb * 128:(jb + 1) * 128],
                               wvT[:, dc, n0:n1],
                               start=(dc == 0), stop=(dc == DC - 1))
                    nc.vector.tensor_copy(vV[:, jb, 0:D], pv)
                    nc.vector.tensor_copy(vV[:, jb, D:D + 2], ones1)

            # ---- Attention (LAG-pipelined)
            LAG = 2
            sched = [(u, jj) for u in range(NPAIR) for jj in range(4 * u + 4)]
            with (
                tc.tile_pool(name="expp", bufs=4) as expp,
                tc.tile_pool(name="outp", bufs=3) as outp,
                tc.tile_pool(name="ps_av", bufs=1, space="PSUM") as ps_av,
                tc.tile_pool(name="ps_s", bufs=4, space="PSUM") as ps_s,
            ):
                av_tiles = {}
                pending = []

                def emit_scores(u, jj):
                    ps = ps_s.tile([128, 256], f32, tag="ps", name=f"ps{u}_{jj}")
                    for oc in range(OC):
                        mm(ps, xT[:, oc, jj * 128:(jj + 1) * 128],
                           gqT[:, oc, u * 256:(u + 1) * 256],
                           start=(oc == 0), stop=(oc == OC - 1))
                    m = jj - 4 * u
                    if m >= 0:
                        nc.vector.tensor_add(ps, ps, masks[:, m, :])
                    ex = expp.tile([128, 256], bf16, tag="ex", name=f"ex{u}_{jj}")
                    nc.scalar.activation(ex, ps, Exp, scale=SCALE)
                    return (u, jj, ex)

                def emit_av(u, jj, ex):
                    njb = 4 * u + 4
                    if jj == 0:
                        av_tiles[u] = [
                            ps_av.tile([128, D + 2], f32, tag=f"av{g}",
                                       name=f"av{u}_{g}")
                            for g in (0, 1)
                        ]
                    av = av_tiles[u]
                    for g in (0, 1):
                        for n0, n1 in ((0, 512), (512, D + 2)):
                            mm(av[g][:, n0:n1], ex[:, g * 128:(g + 1) * 128],
                               vV[:, jj, n0:n1],
                               start=(jj == 0), stop=(jj == njb - 1))
                    if jj == njb - 1:
                        for g in (0, 1):
                            rec = outp.tile([128, 1], f32, tag="rec",
                                            name=f"rec{u}_{g}")
                            nc.vector.reciprocal(rec, av[g][:, D:D + 1])
                            ot = outp.tile([128, D], bf16, tag="ot",
                                           name=f"ot{u}_{g}")
                            nc.scalar.mul(ot, av[g][:, 0:D], rec)
                            r0 = (2 * u + g) * 128
                            nc.default_dma_engine.dma_start(
                                out=out_d[r0:r0 + 128, :], in_=ot
                            )
                        del av_tiles[u]

                for idx in range(len(sched) + LAG):
                    if idx < len(sched):
                        pending.append(emit_scores(*sched[idx]))
                    if idx >= LAG:
                        emit_av(*pending.pop(0))
    nc.finalize()
    return nc


def _build_masks(a: int) -> np.ndarray:
    """Additive pre-softmax masks for the last 4 permuted j-positions of each
    pair. Query pair u = globals {4u+2a, 4u+2a+1} at permuted positions
    {4u+2, 4u+3}; positions {4u, 4u+1} hold globals {4u+2-2a, 4u+3-2a}."""
    keep = np.triu(np.ones((128, 128), dtype=bool))  # keep iff k(p) <= q(f)
    P0 = np.zeros((128, 256), dtype=np.float32)
    P1 = np.zeros((128, 256), dtype=np.float32)
    P1[:, :128] = np.where(keep, 0.0, NEG)
    P2 = np.full((128, 256), NEG, dtype=np.float32)
    P2[:, 128:] = np.where(keep, 0.0, NEG)
    P3 = np.full((128, 256), NEG, dtype=np.float32)
    if a == 0:
        return np.stack([P3, P3, P1, P2])
    return np.stack([P0, P0, P1, P2])


def _local_blocks(a: int):
    """Global 128-row block index for each local query block L = 0..15."""
    return [4 * (L // 2) + 2 * a + (L % 2) for L in range(16)]


def _col_perm(a: int):
    """Permuted column-block order: group u = [other pair, own pair]."""
    perm = []
    for u in range(NPAIR):
        if a == 0:
            perm += [4 * u + 2, 4 * u + 3, 4 * u, 4 * u + 1]
        else:
            perm += [4 * u, 4 * u + 1, 4 * u + 2, 4 * u + 3]
    return perm


def build_in_maps(x, W_q, W_k, W_v):
    import ml_dtypes

    bf16 = ml_dtypes.bfloat16
    x = np.asarray(x, dtype=np.float32)
    Wq = np.asarray(W_q, dtype=np.float32)
    Wk = np.asarray(W_k, dtype=np.float32)
    Wv = np.asarray(W_v, dtype=np.float32)

    m = np.ascontiguousarray(
        (Wq.T.astype(np.float64) @ Wk.astype(np.float64)).astype(np.float32)
    ).astype(bf16)                                   # [d, o]
    wvT = np.ascontiguousarray(Wv.T).astype(bf16)    # [d, o]
    masks = [
        np.ascontiguousarray(_build_masks(a).transpose(1, 0, 2)).astype(bf16)
        for a in (0, 1)
    ]                                                # [128, 4, 256]

    in_maps = []
    for c in range(8):
        b, a = c % 4, c // 4
        xTb = np.ascontiguousarray(x[b].T).astype(bf16)   # [D, T]
        xTp = np.ascontiguousarray(
            xTb.reshape(D, NJB, 128)[:, _col_perm(a), :].reshape(D, T)
        )
        in_maps.append(
            {"xT": xTp, "m": m, "wvT": wvT, "masks": masks[a]}
        )
    return in_maps


def last_in_maps(inputs):
    return build_in_maps(
        inputs["x"], inputs["W_q"], inputs["W_k"], inputs["W_v"]
    )


def kernel(x, W_q, W_k, W_v):
    global _COMPILED
    from concourse.bass_utils import run_bass_kernel_spmd

    if _COMPILED is None:
        _COMPILED = build_program()
    nc = _COMPILED

    in_maps = build_in_maps(x, W_q, W_k, W_v)
    res = run_bass_kernel_spmd(nc, in_maps, list(range(8)))
    out = np.empty((B, T, D), dtype=np.float32)
    for c in range(8):
        b, a = c % 4, c // 4
        oc_loc = np.asarray(res.results[c]["out"]).astype(np.float32)
        for L, gb in enumerate(_local_blocks(a)):
            out[b, gb * 128:(gb + 1) * 128] = oc_loc[L * 128:(L + 1) * 128]
    return out
